# revision 1
# baseline (speedup 1.0000x reference)
"""Band VQ forward on 8 Trainium2 NeuronCores.

Problem: x [B=8, NB=3, D=512, T=2048] f32, codebook [NB=3, K=1024, D=512] f32.
Returns (quantized [B,NB,D,T] f32, codes [B,NB,T] int32, commit_loss scalar).

Sharding: data-parallel over batch B -> one batch per core; codebooks
replicated on every core; commit loss reduced from per-core partials on host.

Per-core device pipeline (per band, per 128-frame tile):
  - scores[t,k] = 2*x.e  via PE matmuls. fp32 accuracy at tf32 speed using a
    3-term hi/lo split (a_hi*b_hi + a_hi*b_lo + a_lo*b_hi) in float32r mode
    (1 cyc/row vs 4 for fp32; validated: max err ~1e-6 rel, better than
    single-rounded fp32).
  - scores -= |e|^2 (broadcast, fused into the PSUM->SBUF copy on DVE)
  - argmax via DVE max/max_index (argmin of distance = argmax of score)
  - gather codeword rows with indirect DMA, PE-transpose to channels-first
  - commit loss via identity sum((q-x)^2) = sum(x^2) - sum(max_score):
    sum(x^2) is computed host-side in f64 during input prep; max scores are
    shipped back per frame.
"""
import numpy as np
from contextlib import ExitStack

import concourse.bass as bass
import concourse.tile as tile
from concourse import bacc, mybir
from concourse import bass_utils
from concourse.masks import make_identity

B, NB, D, T, K = 8, 3, 512, 2048, 1024
P = 128
NDC = D // P        # 4 d-chunks of 128
TCH = 512           # t-chunk (diff/output granularity)
KC = 512            # k half (one PSUM bank)

f32 = mybir.dt.float32
f32r = mybir.dt.float32r
u32 = mybir.dt.uint32
i32 = mybir.dt.int32

TRACE = False        # test.py can flip this to capture an NTFF profile
LAST = {}            # test.py introspection (exec_time_ns etc.)


def tf32_split(a: np.ndarray):
    """Round-to-nearest split of fp32 into (hi, lo): hi has a 10-bit (tf32)
    mantissa, lo = a - hi exactly. hi + lo == a exactly."""
    u = np.ascontiguousarray(a).view(np.uint32)
    r = (u + np.uint32(0x0FFF) + ((u >> np.uint32(13)) & np.uint32(1))) & np.uint32(
        0xFFFFE000
    )
    hi = r.view(np.float32)
    lo = (a - hi).astype(np.float32)
    return hi, lo


def build_nc(t_total: int = T):
    """Build the per-core Bass program. t_total lets tests build a smaller
    variant for CoreSim."""
    ntc = t_total // TCH      # t-chunks
    ntt = TCH // P            # 128-frame tiles per chunk

    nc = bacc.Bacc("TRN2", target_bir_lowering=False, debug=False)

    xh_d = nc.dram_tensor("xh", [NB, D, t_total], f32r, kind="ExternalInput").ap()
    xl_d = nc.dram_tensor("xl", [NB, D, t_total], f32r, kind="ExternalInput").ap()
    bh_d = nc.dram_tensor("bh", [NB, D, K], f32r, kind="ExternalInput").ap()
    bl_d = nc.dram_tensor("bl", [NB, D, K], f32r, kind="ExternalInput").ap()
    e2_d = nc.dram_tensor("e2", [NB, K], f32, kind="ExternalInput").ap()
    cb_d = [
        nc.dram_tensor(f"cb{n}", [K, D], f32, kind="ExternalInput").ap()
        for n in range(NB)
    ]

    q_d = nc.dram_tensor("q", [NB, D, t_total], f32, kind="ExternalOutput").ap()
    codes_d = nc.dram_tensor("codes", [NB, t_total], i32, kind="ExternalOutput").ap()
    smax_d = nc.dram_tensor("smax", [NB, t_total], f32, kind="ExternalOutput").ap()

    xh_r = xh_d.rearrange("n (c p) t -> p n c t", p=P)
    xl_r = xl_d.rearrange("n (c p) t -> p n c t", p=P)
    q_r = q_d.rearrange("n (c p) t -> p n c t", p=P)

    with tile.TileContext(nc) as tc, ExitStack() as ctx:
        const = ctx.enter_context(tc.tile_pool(name="const", bufs=1))
        bpool = ctx.enter_context(tc.tile_pool(name="bpool", bufs=2))
        xpool = ctx.enter_context(tc.tile_pool(name="xpool", bufs=3))
        qpool = ctx.enter_context(tc.tile_pool(name="qpool", bufs=2))
        scpool = ctx.enter_context(tc.tile_pool(name="scpool", bufs=3))
        qrpool = ctx.enter_context(tc.tile_pool(name="qrpool", bufs=3))
        mpool = ctx.enter_context(tc.tile_pool(name="mpool", bufs=4))
        pspool = ctx.enter_context(tc.tile_pool(name="pspool", bufs=3, space="PSUM"))
        pqpool = ctx.enter_context(tc.tile_pool(name="pqpool", bufs=2, space="PSUM"))

        ident = const.tile([P, P], f32)
        make_identity(nc, ident[:])

        e2bc = const.tile([P, NB, K], f32)
        nc.sync.dma_start(e2bc[:], e2_d[None].to_broadcast([P, NB, K]))

        for band in range(NB):
            bh_t = bpool.tile([P, NDC, K], f32r, tag="bh")
            bl_t = bpool.tile([P, NDC, K], f32r, tag="bl")
            nc.sync.dma_start(bh_t[:], bh_d[band].rearrange("(c p) k -> p c k", p=P))
            nc.sync.dma_start(bl_t[:], bl_d[band].rearrange("(c p) k -> p c k", p=P))

            for tci in range(ntc):
                t0 = tci * TCH
                xh_t = xpool.tile([P, NDC, TCH], f32r, tag="xh")
                xl_t = xpool.tile([P, NDC, TCH], f32r, tag="xl")
                nc.sync.dma_start(xh_t[:], xh_r[:, band, :, t0 : t0 + TCH])
                nc.sync.dma_start(xl_t[:], xl_r[:, band, :, t0 : t0 + TCH])

                q_tc = qpool.tile([P, NDC, TCH], f32, tag="qtc")

                for tt in range(ntt):
                    tt0 = tt * P
                    ps = pspool.tile([P, K], f32, tag="ps")
                    for dc in range(NDC):
                        lh = xh_t[:, dc, tt0 : tt0 + P]
                        ll = xl_t[:, dc, tt0 : tt0 + P]
                        st = dc == 0
                        sp = dc == NDC - 1
                        nc.tensor.matmul(ps[:, 0:KC], lh, bh_t[:, dc, 0:KC],
                                         start=st, stop=False)
                        nc.tensor.matmul(ps[:, KC:K], lh, bh_t[:, dc, KC:K],
                                         start=st, stop=False)
                        nc.tensor.matmul(ps[:, 0:KC], lh, bl_t[:, dc, 0:KC],
                                         start=False, stop=False)
                        nc.tensor.matmul(ps[:, KC:K], lh, bl_t[:, dc, KC:K],
                                         start=False, stop=False)
                        nc.tensor.matmul(ps[:, 0:KC], ll, bh_t[:, dc, 0:KC],
                                         start=False, stop=sp)
                        nc.tensor.matmul(ps[:, KC:K], ll, bh_t[:, dc, KC:K],
                                         start=False, stop=sp)

                    sc = scpool.tile([P, K], f32, tag="sc")
                    nc.vector.tensor_tensor(
                        out=sc[:], in0=ps[:], in1=e2bc[:, band, :],
                        op=mybir.AluOpType.subtract,
                    )
                    mx = mpool.tile([P, 8], f32, tag="mx")
                    mi = mpool.tile([P, 8], u32, tag="mi")
                    nc.vector.max(mx[:], sc[:])
                    nc.vector.max_index(mi[:], mx[:], sc[:])

                    tg = t0 + tt0
                    nc.sync.dma_start(
                        codes_d[band, tg : tg + P, None], mi[:, 0:1].bitcast(i32)
                    )
                    nc.sync.dma_start(smax_d[band, tg : tg + P, None], mx[:, 0:1])

                    qr = qrpool.tile([P, D], f32, tag="qr")
                    nc.gpsimd.indirect_dma_start(
                        out=qr[:],
                        out_offset=None,
                        in_=cb_d[band],
                        in_offset=bass.IndirectOffsetOnAxis(ap=mi[:, 0:1], axis=0),
                    )

                    pq = pqpool.tile([P, D], f32, tag="pq")
                    for c in range(NDC):
                        nc.tensor.transpose(
                            pq[:, c * P : (c + 1) * P],
                            qr[:, c * P : (c + 1) * P],
                            ident[:],
                        )
                    nc.scalar.copy(
                        out=q_tc[:, :, tt0 : tt0 + P],
                        in_=pq[:].rearrange("p (c t) -> p c t", c=NDC),
                    )

                nc.sync.dma_start(q_r[:, band, :, t0 : t0 + TCH], q_tc[:])

    nc.compile()
    return nc


_NC_CACHE = {}


def _get_nc(t_total: int = T):
    if t_total not in _NC_CACHE:
        _NC_CACHE[t_total] = build_nc(t_total)
    return _NC_CACHE[t_total]


def prep_inputs(x: np.ndarray, codebook: np.ndarray):
    """Host-side prep: hi/lo splits, |e|^2, per-core input maps, sum(x^2)."""
    x = np.ascontiguousarray(x, dtype=np.float32)
    codebook = np.ascontiguousarray(codebook, dtype=np.float32)

    cbt2 = np.ascontiguousarray(2.0 * codebook.transpose(0, 2, 1))  # [NB, D, K]
    bh, bl = tf32_split(cbt2)
    e2 = (codebook.astype(np.float64) ** 2).sum(-1).astype(np.float32)  # [NB, K]

    shared = {"bh": bh, "bl": bl, "e2": e2}
    for n in range(NB):
        shared[f"cb{n}"] = np.ascontiguousarray(codebook[n])

    in_maps = []
    for b in range(B):
        xh, xl = tf32_split(x[b])
        in_maps.append({"xh": xh, "xl": xl, **shared})

    sum_x2 = float((x.astype(np.float64) ** 2).sum())
    return in_maps, sum_x2


def kernel(x: np.ndarray, codebook: np.ndarray):
    x = np.asarray(x)
    codebook = np.asarray(codebook)
    in_maps, sum_x2 = prep_inputs(x, codebook)
    nc = _get_nc()

    res = bass_utils.run_bass_kernel_spmd(
        nc, in_maps, core_ids=list(range(B)), trace=TRACE
    )
    LAST["exec_time_ns"] = res.exec_time_ns
    LAST["profile_json"] = res.profile_json

    outs = res.results
    quantized = np.stack([outs[b]["q"] for b in range(B)])        # [B,NB,D,T]
    codes = np.stack([outs[b]["codes"] for b in range(B)])        # [B,NB,T]
    smax = np.stack([outs[b]["smax"] for b in range(B)])          # [B,NB,T]

    sum_smax = float(smax.astype(np.float64).sum())
    commit_loss = np.float32((sum_x2 - sum_smax) / (B * NB * T * D))
    return quantized, codes, commit_loss


# revision 4
# speedup vs baseline: 1.4395x; 1.4395x over previous
"""Band VQ forward on 8 Trainium2 NeuronCores.

Problem: x [B=8, NB=3, D=512, T=2048] f32, codebook [NB=3, K=1024, D=512] f32.
Returns (quantized [B,NB,D,T] f32, codes [B,NB,T] int32, commit_loss scalar).

Sharding: data-parallel over batch B -> one batch per core; codebooks
replicated on every core; commit loss reduced from per-core partials on host.

Per-core device pipeline (per band, per 128-frame tile):
  - scores[t,k] = (2x).e  via PE matmuls. fp32 accuracy at tf32 speed using a
    3-term hi/lo split (a_hi*b_hi + a_hi*b_lo + a_lo*b_hi) in float32r mode
    (1 cyc/row vs 4 for fp32; measured max err ~1e-6 rel, better than
    single-rounded fp32). The 2x scale is folded into the x split host-side.
  - scores -= |e|^2 broadcast (DVE), argmax via DVE max/max_index
    (argmin of distance = argmax of score)
  - gather codeword rows with indirect DMA, PE-transpose to channels-first.
    Transposes/copies for tile i are emitted after the matmuls of tile i+2 so
    the PE never stalls on the argmax->gather chain.
  - commit loss via identity sum((q-x)^2) = sum(x^2) - sum(max_score):
    sum(x^2) in f64 on host during input prep; per-frame max scores shipped
    back.
"""
import numpy as np
from contextlib import ExitStack

import concourse.bass as bass
import concourse.tile as tile
from concourse import bacc, mybir
from concourse import bass_utils
from concourse.masks import make_identity

B, NB, D, T, K = 8, 3, 512, 2048, 1024
P = 128
NDC = D // P        # 4 d-chunks of 128
TCH = 512           # t-chunk (q assembly/output granularity)
XCH = 1024          # x load chunk (4KB contiguous runs)
KC = 512            # k half (one PSUM bank)
DEFER = 2           # tiles of slack between matmuls and PE transposes

f32 = mybir.dt.float32
f32r = mybir.dt.float32r
u32 = mybir.dt.uint32
i32 = mybir.dt.int32

TRACE = False        # test.py can flip this to capture an NTFF profile
LAST = {}            # test.py introspection (exec_time_ns etc.)


def tf32_split(a: np.ndarray):
    """Round-to-nearest split of fp32 into (hi, lo): hi has a 10-bit (tf32)
    mantissa, lo = a - hi exactly. hi + lo == a exactly."""
    u = np.ascontiguousarray(a).view(np.uint32)
    r = (u + np.uint32(0x0FFF) + ((u >> np.uint32(13)) & np.uint32(1))) & np.uint32(
        0xFFFFE000
    )
    hi = r.view(np.float32)
    lo = (a - hi).astype(np.float32)
    return hi, lo


def build_nc(t_total: int = T):
    """Build the per-core Bass program. t_total lets tests build a smaller
    variant for CoreSim."""
    ntc = t_total // TCH      # t-chunks per band
    ntt = TCH // P            # 128-frame tiles per chunk
    xch = min(XCH, t_total)   # x load chunk
    nxc = t_total // xch      # x-load chunks per band

    nc = bacc.Bacc("TRN2", target_bir_lowering=False, debug=False)

    xh_d = nc.dram_tensor("xh", [NB, D, t_total], f32r, kind="ExternalInput").ap()
    xl_d = nc.dram_tensor("xl", [NB, D, t_total], f32r, kind="ExternalInput").ap()
    bh_d = nc.dram_tensor("bh", [NB, D, K], f32r, kind="ExternalInput").ap()
    bl_d = nc.dram_tensor("bl", [NB, D, K], f32r, kind="ExternalInput").ap()
    e2_d = nc.dram_tensor("e2", [NB, K], f32, kind="ExternalInput").ap()
    cb_d = [
        nc.dram_tensor(f"cb{n}", [K, D], f32, kind="ExternalInput").ap()
        for n in range(NB)
    ]

    q_d = nc.dram_tensor("q", [NB, D, t_total], f32, kind="ExternalOutput").ap()
    codes_d = nc.dram_tensor("codes", [NB, t_total], i32, kind="ExternalOutput").ap()
    smax_d = nc.dram_tensor("smax", [NB, t_total], f32, kind="ExternalOutput").ap()

    xh_r = xh_d.rearrange("n (c p) t -> p n c t", p=P)
    xl_r = xl_d.rearrange("n (c p) t -> p n c t", p=P)
    q_r = q_d.rearrange("n (c p) t -> p n c t", p=P)

    with tile.TileContext(nc) as tc, ExitStack() as ctx:
        const = ctx.enter_context(tc.tile_pool(name="const", bufs=1))
        bpool = ctx.enter_context(tc.tile_pool(name="bpool", bufs=2))
        xpool = ctx.enter_context(tc.tile_pool(name="xpool", bufs=2))
        qpool = ctx.enter_context(tc.tile_pool(name="qpool", bufs=3))
        scpool = ctx.enter_context(tc.tile_pool(name="scpool", bufs=3))
        qrpool = ctx.enter_context(tc.tile_pool(name="qrpool", bufs=4))
        mpool = ctx.enter_context(tc.tile_pool(name="mpool", bufs=6))
        pspool = ctx.enter_context(tc.tile_pool(name="pspool", bufs=3, space="PSUM"))
        pqpool = ctx.enter_context(tc.tile_pool(name="pqpool", bufs=2, space="PSUM"))

        ident = const.tile([P, P], f32)
        make_identity(nc, ident[:])

        e2bc = const.tile([P, NB, K], f32)
        nc.sync.dma_start(e2bc[:], e2_d[None].to_broadcast([P, NB, K]))

        q_tiles = {}      # (band, tci) -> (tile, writes_done)
        pending = []      # deferred (band, tci, tt, qr_tile)

        def flush_one():
            band, tci, tt, qr = pending.pop(0)
            key = (band, tci)
            if key not in q_tiles:
                q_tiles[key] = [
                    qpool.tile([P, NDC, TCH], f32, tag="qtc",
                               name=f"qtc_{band}_{tci}"), 0]
            q_tc, done = q_tiles[key]
            pq = pqpool.tile([P, D], f32, tag="pq")
            for c in range(NDC):
                nc.tensor.transpose(
                    pq[:, c * P : (c + 1) * P],
                    qr[:, c * P : (c + 1) * P],
                    ident[:],
                )
            tt0 = tt * P
            nc.scalar.copy(
                out=q_tc[:, :, tt0 : tt0 + P],
                in_=pq[:].rearrange("p (c t) -> p c t", c=NDC),
            )
            q_tiles[key][1] = done + 1
            if q_tiles[key][1] == ntt:
                t0 = tci * TCH
                nc.sync.dma_start(q_r[:, band, :, t0 : t0 + TCH], q_tc[:])
                del q_tiles[key]

        for band in range(NB):
            bh_t = bpool.tile([P, NDC, K], f32r, tag="bh")
            bl_t = bpool.tile([P, NDC, K], f32r, tag="bl")
            nc.sync.dma_start(bh_t[:], bh_d[band].rearrange("(c p) k -> p c k", p=P))
            nc.sync.dma_start(bl_t[:], bl_d[band].rearrange("(c p) k -> p c k", p=P))

            for xci in range(nxc):
                x0 = xci * xch
                xh_t = xpool.tile([P, NDC, xch], f32r, tag="xh")
                xl_t = xpool.tile([P, NDC, xch], f32r, tag="xl")
                nc.sync.dma_start(xh_t[:], xh_r[:, band, :, x0 : x0 + xch])
                nc.sync.dma_start(xl_t[:], xl_r[:, band, :, x0 : x0 + xch])

                for tti in range(xch // P):          # 128-frame tiles in x chunk
                    tg = x0 + tti * P                # global t offset
                    tci = tg // TCH
                    tt = (tg % TCH) // P
                    tt0 = tti * P

                    ps = pspool.tile([P, K], f32, tag="ps")
                    for dc in range(NDC):
                        lh = xh_t[:, dc, tt0 : tt0 + P]
                        ll = xl_t[:, dc, tt0 : tt0 + P]
                        st = dc == 0
                        sp = dc == NDC - 1
                        nc.tensor.matmul(ps[:, 0:KC], lh, bh_t[:, dc, 0:KC],
                                         start=st, stop=False)
                        nc.tensor.matmul(ps[:, KC:K], lh, bh_t[:, dc, KC:K],
                                         start=st, stop=False)
                        nc.tensor.matmul(ps[:, 0:KC], lh, bl_t[:, dc, 0:KC],
                                         start=False, stop=False)
                        nc.tensor.matmul(ps[:, KC:K], lh, bl_t[:, dc, KC:K],
                                         start=False, stop=False)
                        nc.tensor.matmul(ps[:, 0:KC], ll, bh_t[:, dc, 0:KC],
                                         start=False, stop=sp)
                        nc.tensor.matmul(ps[:, KC:K], ll, bh_t[:, dc, KC:K],
                                         start=False, stop=sp)

                    sc = scpool.tile([P, K], f32, tag="sc")
                    nc.vector.tensor_tensor(
                        out=sc[:], in0=ps[:], in1=e2bc[:, band, :],
                        op=mybir.AluOpType.subtract,
                    )
                    mx = mpool.tile([P, 8], f32, tag="mx")
                    mi = mpool.tile([P, 8], u32, tag="mi")
                    nc.vector.max(mx[:], sc[:])
                    nc.vector.max_index(mi[:], mx[:], sc[:])

                    nc.sync.dma_start(
                        codes_d[band, tg : tg + P, None], mi[:, 0:1].bitcast(i32)
                    )
                    nc.sync.dma_start(smax_d[band, tg : tg + P, None], mx[:, 0:1])

                    qr = qrpool.tile([P, D], f32, tag="qr")
                    nc.gpsimd.indirect_dma_start(
                        out=qr[:],
                        out_offset=None,
                        in_=cb_d[band],
                        in_offset=bass.IndirectOffsetOnAxis(ap=mi[:, 0:1], axis=0),
                    )

                    pending.append((band, tci, tt, qr))
                    if len(pending) > DEFER:
                        flush_one()

        while pending:
            flush_one()

    nc.compile()
    return nc


_NC_CACHE = {}


def _get_nc(t_total: int = T):
    if t_total not in _NC_CACHE:
        _NC_CACHE[t_total] = build_nc(t_total)
    return _NC_CACHE[t_total]


def prep_inputs(x: np.ndarray, codebook: np.ndarray):
    """Host-side prep: hi/lo splits, |e|^2, per-core input maps, sum(x^2)."""
    x = np.ascontiguousarray(x, dtype=np.float32)
    codebook = np.ascontiguousarray(codebook, dtype=np.float32)

    cbt = np.ascontiguousarray(codebook.transpose(0, 2, 1))  # [NB, D, K]
    bh, bl = tf32_split(cbt)
    e2 = (codebook.astype(np.float64) ** 2).sum(-1).astype(np.float32)  # [NB, K]

    shared = {"bh": bh, "bl": bl, "e2": e2}
    for n in range(NB):
        shared[f"cb{n}"] = np.ascontiguousarray(codebook[n])

    in_maps = []
    for b in range(B):
        xh, xl = tf32_split(2.0 * x[b])
        in_maps.append({"xh": xh, "xl": xl, **shared})

    sum_x2 = float((x.astype(np.float64) ** 2).sum())
    return in_maps, sum_x2


def kernel(x: np.ndarray, codebook: np.ndarray):
    x = np.asarray(x)
    codebook = np.asarray(codebook)
    in_maps, sum_x2 = prep_inputs(x, codebook)
    nc = _get_nc()

    res = bass_utils.run_bass_kernel_spmd(
        nc, in_maps, core_ids=list(range(B)), trace=TRACE
    )
    LAST["exec_time_ns"] = res.exec_time_ns
    LAST["profile_json"] = res.profile_json

    outs = res.results
    quantized = np.stack([outs[b]["q"] for b in range(B)])        # [B,NB,D,T]
    codes = np.stack([outs[b]["codes"] for b in range(B)])        # [B,NB,T]
    smax = np.stack([outs[b]["smax"] for b in range(B)])          # [B,NB,T]

    sum_smax = float(smax.astype(np.float64).sum())
    commit_loss = np.float32((sum_x2 - sum_smax) / (B * NB * T * D))
    return quantized, codes, commit_loss


# revision 7
# speedup vs baseline: 1.5596x; 1.0834x over previous
"""Band VQ forward on 8 Trainium2 NeuronCores.

Problem: x [B=8, NB=3, D=512, T=2048] f32, codebook [NB=3, K=1024, D=512] f32.
Returns (quantized [B,NB,D,T] f32, codes [B,NB,T] int32, commit_loss scalar).

Sharding: data-parallel over batch B -> one batch per core; codebooks
replicated on every core; commit loss reduced from per-core partials on host.

Per-core device pipeline (per band, per 128-frame tile):
  - scores[t,k] = (2x).e  via PE matmuls. fp32 accuracy at tf32 speed using a
    3-term hi/lo split (a_hi*b_hi + a_hi*b_lo + a_lo*b_hi) in float32r mode
    (1 cyc/row vs 4 for fp32; measured ~1e-6 rel err). The hi/lo split is done
    ON DEVICE (f32->f32r cast rounds to the 11-bit tf32 mantissa; lo = x - hi)
    so x and the codebook ship over DMA once in plain fp32. The 2x scale is
    folded into x host-side.
  - scores -= |e|^2 broadcast (DVE), argmax via DVE max/max_index
    (argmin of distance = argmax of score)
  - gather codeword rows with indirect DMA, PE-transpose to channels-first.
    Transposes/copies for tile i are emitted after the matmuls of tile i+DEFER
    so the PE never stalls on the argmax->gather chain.
  - commit loss via identity sum((q-x)^2) = sum(x^2) - sum(max_score):
    sum(x^2) in f64 on host during input prep; per-frame max scores shipped
    back.
"""
import numpy as np
from contextlib import ExitStack

import concourse.bass as bass
import concourse.tile as tile
from concourse import bacc, mybir
from concourse import bass_utils
from concourse.masks import make_identity

B, NB, D, T, K = 8, 3, 512, 2048, 1024
P = 128
NDC = D // P        # 4 d-chunks of 128
TCH = 1024          # q assembly/output chunk (4KB contiguous output runs)
XCH = 512           # x load/split chunk
KC = 512            # k half (one PSUM bank)
DEFER = 4           # tiles of slack between matmuls and PE transposes

f32 = mybir.dt.float32
f32r = mybir.dt.float32r
u32 = mybir.dt.uint32
i32 = mybir.dt.int32

TRACE = False        # test.py can flip this to capture an NTFF profile
LAST = {}            # test.py introspection (exec_time_ns etc.)


def build_nc(t_total: int = T):
    """Build the per-core Bass program. t_total lets tests build a smaller
    variant for CoreSim."""
    tch = min(TCH, t_total)
    ntt = tch // P            # 128-frame tiles per q chunk
    xch = min(XCH, t_total)   # x load chunk
    nxc = t_total // xch

    nc = bacc.Bacc("TRN2", target_bir_lowering=False, debug=False)

    x_d = nc.dram_tensor("x2", [NB, D, t_total], f32, kind="ExternalInput").ap()
    b_d = nc.dram_tensor("bt", [NB, D, K], f32, kind="ExternalInput").ap()
    e2_d = nc.dram_tensor("e2", [NB, K], f32, kind="ExternalInput").ap()
    cb_d = [
        nc.dram_tensor(f"cb{n}", [K, D], f32, kind="ExternalInput").ap()
        for n in range(NB)
    ]

    q_d = nc.dram_tensor("q", [NB, D, t_total], f32, kind="ExternalOutput").ap()
    codes_d = nc.dram_tensor("codes", [NB, t_total], i32, kind="ExternalOutput").ap()
    smax_d = nc.dram_tensor("smax", [NB, t_total], f32, kind="ExternalOutput").ap()

    x_r = x_d.rearrange("n (c p) t -> p n c t", p=P)
    q_r = q_d.rearrange("n (c p) t -> p n c t", p=P)

    with tile.TileContext(nc) as tc, ExitStack() as ctx:
        const = ctx.enter_context(tc.tile_pool(name="const", bufs=1))
        bspool = ctx.enter_context(tc.tile_pool(name="bspool", bufs=1))
        bpool = ctx.enter_context(tc.tile_pool(name="bpool", bufs=2))
        xspool = ctx.enter_context(tc.tile_pool(name="xspool", bufs=2))
        xpool = ctx.enter_context(tc.tile_pool(name="xpool", bufs=2))
        qpool = ctx.enter_context(tc.tile_pool(name="qpool", bufs=2))
        scpool = ctx.enter_context(tc.tile_pool(name="scpool", bufs=2))
        qrpool = ctx.enter_context(tc.tile_pool(name="qrpool", bufs=6))
        mpool = ctx.enter_context(tc.tile_pool(name="mpool", bufs=8))
        pspool = ctx.enter_context(tc.tile_pool(name="pspool", bufs=3, space="PSUM"))
        pqpool = ctx.enter_context(tc.tile_pool(name="pqpool", bufs=2, space="PSUM"))

        ident = const.tile([P, P], f32)
        make_identity(nc, ident[:])

        e2bc = const.tile([P, NB, K], f32)
        nc.sync.dma_start(e2bc[:], e2_d[None].to_broadcast([P, NB, K]))

        q_tiles = {}      # (band, tci) -> [tile, writes_done]
        pending = []      # deferred (band, tci, tt, qr_tile)

        def flush_one():
            band, tci, tt, qr = pending.pop(0)
            key = (band, tci)
            if key not in q_tiles:
                q_tiles[key] = [
                    qpool.tile([P, NDC, tch], f32, tag="qtc",
                               name=f"qtc_{band}_{tci}"), 0]
            q_tc, done = q_tiles[key]
            pq = pqpool.tile([P, D], f32, tag="pq", name=f"pq_{band}_{tci}_{tt}")
            for c in range(NDC):
                nc.tensor.transpose(
                    pq[:, c * P : (c + 1) * P],
                    qr[:, c * P : (c + 1) * P],
                    ident[:],
                )
            tt0 = tt * P
            nc.scalar.copy(
                out=q_tc[:, :, tt0 : tt0 + P],
                in_=pq[:].rearrange("p (c t) -> p c t", c=NDC),
            )
            q_tiles[key][1] = done + 1
            if q_tiles[key][1] == ntt:
                t0 = tci * tch
                nc.sync.dma_start(q_r[:, band, :, t0 : t0 + tch], q_tc[:])
                del q_tiles[key]

        for band in range(NB):
            # load codebook fp32, split hi/lo on device
            bfp = bspool.tile([P, NDC, K], f32, tag="bfp")
            nc.sync.dma_start(
                bfp[:], b_d[band].rearrange("(c p) k -> p c k", p=P)
            )
            bh_t = bpool.tile([P, NDC, K], f32r, tag="bh")
            nc.vector.tensor_copy(bh_t[:], bfp[:])
            bl_t = bpool.tile([P, NDC, K], f32r, tag="bl")
            nc.vector.tensor_tensor(
                out=bl_t[:], in0=bfp[:], in1=bh_t[:].bitcast(f32),
                op=mybir.AluOpType.subtract,
            )

            for xci in range(nxc):
                x0 = xci * xch
                xfp = xspool.tile([P, NDC, xch], f32, tag="xfp")
                nc.sync.dma_start(xfp[:], x_r[:, band, :, x0 : x0 + xch])
                xh_t = xpool.tile([P, NDC, xch], f32r, tag="xh")
                nc.vector.tensor_copy(xh_t[:], xfp[:])
                xl_t = xpool.tile([P, NDC, xch], f32r, tag="xl")
                nc.vector.tensor_tensor(
                    out=xl_t[:], in0=xfp[:], in1=xh_t[:].bitcast(f32),
                    op=mybir.AluOpType.subtract,
                )

                for tti in range(xch // P):          # 128-frame tiles in x chunk
                    tg = x0 + tti * P                # global t offset
                    tci = tg // tch
                    tt = (tg % tch) // P
                    tt0 = tti * P

                    ps = pspool.tile([P, K], f32, tag="ps")
                    for dc in range(NDC):
                        lh = xh_t[:, dc, tt0 : tt0 + P]
                        ll = xl_t[:, dc, tt0 : tt0 + P]
                        st = dc == 0
                        sp = dc == NDC - 1
                        nc.tensor.matmul(ps[:, 0:KC], lh, bh_t[:, dc, 0:KC],
                                         start=st, stop=False)
                        nc.tensor.matmul(ps[:, KC:K], lh, bh_t[:, dc, KC:K],
                                         start=st, stop=False)
                        nc.tensor.matmul(ps[:, 0:KC], lh, bl_t[:, dc, 0:KC],
                                         start=False, stop=False)
                        nc.tensor.matmul(ps[:, KC:K], lh, bl_t[:, dc, KC:K],
                                         start=False, stop=False)
                        nc.tensor.matmul(ps[:, 0:KC], ll, bh_t[:, dc, 0:KC],
                                         start=False, stop=sp)
                        nc.tensor.matmul(ps[:, KC:K], ll, bh_t[:, dc, KC:K],
                                         start=False, stop=sp)

                    sc = scpool.tile([P, K], f32, tag="sc")
                    nc.vector.tensor_tensor(
                        out=sc[:], in0=ps[:], in1=e2bc[:, band, :],
                        op=mybir.AluOpType.subtract,
                    )
                    mx = mpool.tile([P, 8], f32, tag="mx")
                    mi = mpool.tile([P, 8], u32, tag="mi")
                    nc.vector.max(mx[:], sc[:])
                    nc.vector.max_index(mi[:], mx[:], sc[:])

                    nc.sync.dma_start(
                        codes_d[band, tg : tg + P, None], mi[:, 0:1].bitcast(i32)
                    )
                    nc.sync.dma_start(smax_d[band, tg : tg + P, None], mx[:, 0:1])

                    qr = qrpool.tile([P, D], f32, tag="qr")
                    nc.gpsimd.indirect_dma_start(
                        out=qr[:],
                        out_offset=None,
                        in_=cb_d[band],
                        in_offset=bass.IndirectOffsetOnAxis(ap=mi[:, 0:1], axis=0),
                    )

                    pending.append((band, tci, tt, qr))
                    if len(pending) > DEFER:
                        flush_one()

        while pending:
            flush_one()

    nc.compile()
    return nc


_NC_CACHE = {}


def _get_nc(t_total: int = T):
    if t_total not in _NC_CACHE:
        _NC_CACHE[t_total] = build_nc(t_total)
    return _NC_CACHE[t_total]


def prep_inputs(x: np.ndarray, codebook: np.ndarray):
    """Host-side prep: 2x scale, codebook transpose, |e|^2, per-core maps,
    sum(x^2)."""
    x = np.ascontiguousarray(x, dtype=np.float32)
    codebook = np.ascontiguousarray(codebook, dtype=np.float32)

    cbt = np.ascontiguousarray(codebook.transpose(0, 2, 1))  # [NB, D, K]
    e2 = (codebook.astype(np.float64) ** 2).sum(-1).astype(np.float32)  # [NB, K]

    shared = {"bt": cbt, "e2": e2}
    for n in range(NB):
        shared[f"cb{n}"] = np.ascontiguousarray(codebook[n])

    in_maps = []
    for b in range(B):
        in_maps.append({"x2": 2.0 * x[b], **shared})

    sum_x2 = float((x.astype(np.float64) ** 2).sum())
    return in_maps, sum_x2


def kernel(x: np.ndarray, codebook: np.ndarray):
    x = np.asarray(x)
    codebook = np.asarray(codebook)
    in_maps, sum_x2 = prep_inputs(x, codebook)
    nc = _get_nc()

    res = bass_utils.run_bass_kernel_spmd(
        nc, in_maps, core_ids=list(range(B)), trace=TRACE
    )
    LAST["exec_time_ns"] = res.exec_time_ns
    LAST["profile_json"] = res.profile_json

    outs = res.results
    quantized = np.stack([outs[b]["q"] for b in range(B)])        # [B,NB,D,T]
    codes = np.stack([outs[b]["codes"] for b in range(B)])        # [B,NB,T]
    smax = np.stack([outs[b]["smax"] for b in range(B)])          # [B,NB,T]

    sum_smax = float(smax.astype(np.float64).sum())
    commit_loss = np.float32((sum_x2 - sum_smax) / (B * NB * T * D))
    return quantized, codes, commit_loss


# revision 8
# speedup vs baseline: 2.0674x; 1.3256x over previous
"""Band VQ forward on 8 Trainium2 NeuronCores.

Problem: x [B=8, NB=3, D=512, T=2048] f32, codebook [NB=3, K=1024, D=512] f32.
Returns (quantized [B,NB,D,T] f32, codes [B,NB,T] int32, commit_loss scalar).

Sharding: data-parallel over batch B -> one batch per core; codebooks
replicated on every core; commit loss reduced from per-core partials on host.

Per-core device pipeline (per band, per 128-frame tile):
  - scores[t,k] = (2x).e  via PE matmuls. fp32 accuracy at tf32 speed using a
    3-term hi/lo split (a_hi*b_hi + a_hi*b_lo + a_lo*b_hi) in float32r mode
    (1 cyc/row vs 4 for fp32; measured ~1e-6 rel err). The hi/lo split is done
    ON DEVICE (f32->f32r cast rounds to the 11-bit tf32 mantissa; lo = x - hi)
    so x and the codebook ship over DMA once in plain fp32. The 2x scale is
    folded into x host-side.
  - scores -= |e|^2 broadcast (DVE), argmax via DVE max/max_index
    (argmin of distance = argmax of score)
  - gather codeword rows with indirect DMA, PE-transpose to channels-first.
    Transposes/copies for tile i are emitted after the matmuls of tile i+DEFER
    so the PE never stalls on the argmax->gather chain.
  - commit loss via identity sum((q-x)^2) = sum(x^2) - sum(max_score):
    sum(x^2) in f64 on host during input prep; per-frame max scores shipped
    back.
"""
import numpy as np
from contextlib import ExitStack

import concourse.bass as bass
import concourse.tile as tile
from concourse import bacc, mybir
from concourse import bass_utils
from concourse.masks import make_identity

B, NB, D, T, K = 8, 3, 512, 2048, 1024
P = 128
NDC = D // P        # 4 d-chunks of 128
TCH = 1024          # q assembly/output chunk (4KB contiguous output runs)
XCH = 512           # x load/split chunk
KC = 512            # k half (one PSUM bank)
DEFER = 8           # tiles of slack between matmuls and PE transposes

f32 = mybir.dt.float32
f32r = mybir.dt.float32r
u32 = mybir.dt.uint32
i32 = mybir.dt.int32

TRACE = False        # test.py can flip this to capture an NTFF profile
LAST = {}            # test.py introspection (exec_time_ns etc.)


def build_nc(t_total: int = T):
    """Build the per-core Bass program. t_total lets tests build a smaller
    variant for CoreSim."""
    tch = min(TCH, t_total)
    ntt = tch // P            # 128-frame tiles per q chunk
    xch = min(XCH, t_total)   # x load chunk
    nxc = t_total // xch

    nc = bacc.Bacc("TRN2", target_bir_lowering=False, debug=False)

    x_d = nc.dram_tensor("x2", [NB, D, t_total], f32, kind="ExternalInput").ap()
    b_d = nc.dram_tensor("bt", [NB, D, K], f32, kind="ExternalInput").ap()
    e2_d = nc.dram_tensor("e2", [NB, K], f32, kind="ExternalInput").ap()
    cb_d = [
        nc.dram_tensor(f"cb{n}", [K, D], f32, kind="ExternalInput").ap()
        for n in range(NB)
    ]

    q_d = nc.dram_tensor("q", [NB, D, t_total], f32, kind="ExternalOutput").ap()
    codes_d = nc.dram_tensor("codes", [NB, t_total], i32, kind="ExternalOutput").ap()
    smax_d = nc.dram_tensor("smax", [NB, t_total], f32, kind="ExternalOutput").ap()

    x_r = x_d.rearrange("n (c p) t -> p n c t", p=P)
    q_r = q_d.rearrange("n (c p) t -> p n c t", p=P)

    with tile.TileContext(nc) as tc, ExitStack() as ctx:
        const = ctx.enter_context(tc.tile_pool(name="const", bufs=1))
        bspool = ctx.enter_context(tc.tile_pool(name="bspool", bufs=1))
        bpool = ctx.enter_context(tc.tile_pool(name="bpool", bufs=2))
        xspool = ctx.enter_context(tc.tile_pool(name="xspool", bufs=2))
        xpool = ctx.enter_context(tc.tile_pool(name="xpool", bufs=2))
        qpool = ctx.enter_context(tc.tile_pool(name="qpool", bufs=2))
        scpool = ctx.enter_context(tc.tile_pool(name="scpool", bufs=2))
        qrpool = ctx.enter_context(tc.tile_pool(name="qrpool", bufs=10))
        mpool = ctx.enter_context(tc.tile_pool(name="mpool", bufs=8))
        pspool = ctx.enter_context(tc.tile_pool(name="pspool", bufs=3, space="PSUM"))
        pqpool = ctx.enter_context(tc.tile_pool(name="pqpool", bufs=2, space="PSUM"))

        ident = const.tile([P, P], f32)
        make_identity(nc, ident[:])

        e2bc = const.tile([P, NB, K], f32)
        nc.sync.dma_start(e2bc[:], e2_d[None].to_broadcast([P, NB, K]))

        q_tiles = {}      # (band, tci) -> [tile, writes_done]
        pending = []      # deferred (band, tci, tt, qr_tile)

        def flush_one():
            band, tci, tt, qr = pending.pop(0)
            key = (band, tci)
            if key not in q_tiles:
                q_tiles[key] = [
                    qpool.tile([P, NDC, tch], f32, tag="qtc",
                               name=f"qtc_{band}_{tci}"), 0]
            q_tc, done = q_tiles[key]
            pq = pqpool.tile([P, D], f32, tag="pq", name=f"pq_{band}_{tci}_{tt}")
            for c in range(NDC):
                nc.tensor.transpose(
                    pq[:, c * P : (c + 1) * P],
                    qr[:, c * P : (c + 1) * P],
                    ident[:],
                )
            tt0 = tt * P
            nc.scalar.copy(
                out=q_tc[:, :, tt0 : tt0 + P],
                in_=pq[:].rearrange("p (c t) -> p c t", c=NDC),
            )
            q_tiles[key][1] = done + 1
            if q_tiles[key][1] == ntt:
                t0 = tci * tch
                nc.sync.dma_start(q_r[:, band, :, t0 : t0 + tch], q_tc[:])
                del q_tiles[key]

        for band in range(NB):
            # load codebook fp32, split hi/lo on device
            bfp = bspool.tile([P, NDC, K], f32, tag="bfp")
            nc.sync.dma_start(
                bfp[:], b_d[band].rearrange("(c p) k -> p c k", p=P)
            )
            bh_t = bpool.tile([P, NDC, K], f32r, tag="bh")
            nc.gpsimd.tensor_copy(bh_t[:], bfp[:])
            bl_t = bpool.tile([P, NDC, K], f32r, tag="bl")
            nc.vector.tensor_tensor(
                out=bl_t[:], in0=bfp[:], in1=bh_t[:].bitcast(f32),
                op=mybir.AluOpType.subtract,
            )

            for xci in range(nxc):
                x0 = xci * xch
                xfp = xspool.tile([P, NDC, xch], f32, tag="xfp")
                nc.sync.dma_start(xfp[:], x_r[:, band, :, x0 : x0 + xch])
                xh_t = xpool.tile([P, NDC, xch], f32r, tag="xh")
                nc.scalar.copy(xh_t[:], xfp[:])
                xl_t = xpool.tile([P, NDC, xch], f32r, tag="xl")
                nc.vector.tensor_tensor(
                    out=xl_t[:], in0=xfp[:], in1=xh_t[:].bitcast(f32),
                    op=mybir.AluOpType.subtract,
                )

                for tti in range(xch // P):          # 128-frame tiles in x chunk
                    tg = x0 + tti * P                # global t offset
                    tci = tg // tch
                    tt = (tg % tch) // P
                    tt0 = tti * P

                    ps = pspool.tile([P, K], f32, tag="ps")
                    for dc in range(NDC):
                        lh = xh_t[:, dc, tt0 : tt0 + P]
                        ll = xl_t[:, dc, tt0 : tt0 + P]
                        st = dc == 0
                        sp = dc == NDC - 1
                        nc.tensor.matmul(ps[:, 0:KC], lh, bh_t[:, dc, 0:KC],
                                         start=st, stop=False)
                        nc.tensor.matmul(ps[:, KC:K], lh, bh_t[:, dc, KC:K],
                                         start=st, stop=False)
                        nc.tensor.matmul(ps[:, 0:KC], lh, bl_t[:, dc, 0:KC],
                                         start=False, stop=False)
                        nc.tensor.matmul(ps[:, KC:K], lh, bl_t[:, dc, KC:K],
                                         start=False, stop=False)
                        nc.tensor.matmul(ps[:, 0:KC], ll, bh_t[:, dc, 0:KC],
                                         start=False, stop=sp)
                        nc.tensor.matmul(ps[:, KC:K], ll, bh_t[:, dc, KC:K],
                                         start=False, stop=sp)

                    sc = scpool.tile([P, K], f32, tag="sc")
                    nc.vector.tensor_tensor(
                        out=sc[:], in0=ps[:], in1=e2bc[:, band, :],
                        op=mybir.AluOpType.subtract,
                    )
                    mx = mpool.tile([P, 8], f32, tag="mx")
                    mi = mpool.tile([P, 8], u32, tag="mi")
                    nc.vector.max(mx[:], sc[:])
                    nc.vector.max_index(mi[:], mx[:], sc[:])

                    nc.sync.dma_start(
                        codes_d[band, tg : tg + P, None], mi[:, 0:1].bitcast(i32)
                    )
                    nc.sync.dma_start(smax_d[band, tg : tg + P, None], mx[:, 0:1])

                    qr = qrpool.tile([P, D], f32, tag="qr")
                    nc.gpsimd.indirect_dma_start(
                        out=qr[:],
                        out_offset=None,
                        in_=cb_d[band],
                        in_offset=bass.IndirectOffsetOnAxis(ap=mi[:, 0:1], axis=0),
                    )

                    pending.append((band, tci, tt, qr))
                    if len(pending) > DEFER:
                        flush_one()

        while pending:
            flush_one()

    nc.compile()
    return nc


_NC_CACHE = {}


def _get_nc(t_total: int = T):
    if t_total not in _NC_CACHE:
        _NC_CACHE[t_total] = build_nc(t_total)
    return _NC_CACHE[t_total]


def prep_inputs(x: np.ndarray, codebook: np.ndarray):
    """Host-side prep: 2x scale, codebook transpose, |e|^2, per-core maps,
    sum(x^2)."""
    x = np.ascontiguousarray(x, dtype=np.float32)
    codebook = np.ascontiguousarray(codebook, dtype=np.float32)

    cbt = np.ascontiguousarray(codebook.transpose(0, 2, 1))  # [NB, D, K]
    e2 = (codebook.astype(np.float64) ** 2).sum(-1).astype(np.float32)  # [NB, K]

    shared = {"bt": cbt, "e2": e2}
    for n in range(NB):
        shared[f"cb{n}"] = np.ascontiguousarray(codebook[n])

    in_maps = []
    for b in range(B):
        in_maps.append({"x2": 2.0 * x[b], **shared})

    sum_x2 = float((x.astype(np.float64) ** 2).sum())
    return in_maps, sum_x2


def kernel(x: np.ndarray, codebook: np.ndarray):
    x = np.asarray(x)
    codebook = np.asarray(codebook)
    in_maps, sum_x2 = prep_inputs(x, codebook)
    nc = _get_nc()

    res = bass_utils.run_bass_kernel_spmd(
        nc, in_maps, core_ids=list(range(B)), trace=TRACE
    )
    LAST["exec_time_ns"] = res.exec_time_ns
    LAST["profile_json"] = res.profile_json

    outs = res.results
    quantized = np.stack([outs[b]["q"] for b in range(B)])        # [B,NB,D,T]
    codes = np.stack([outs[b]["codes"] for b in range(B)])        # [B,NB,T]
    smax = np.stack([outs[b]["smax"] for b in range(B)])          # [B,NB,T]

    sum_smax = float(smax.astype(np.float64).sum())
    commit_loss = np.float32((sum_x2 - sum_smax) / (B * NB * T * D))
    return quantized, codes, commit_loss


# revision 9
# speedup vs baseline: 2.1184x; 1.0247x over previous
"""Band VQ forward on 8 Trainium2 NeuronCores.

Problem: x [B=8, NB=3, D=512, T=2048] f32, codebook [NB=3, K=1024, D=512] f32.
Returns (quantized [B,NB,D,T] f32, codes [B,NB,T] int32, commit_loss scalar).

Sharding: data-parallel over batch B -> one batch per core; codebooks
replicated on every core; commit loss reduced from per-core partials on host.

Per-core device pipeline (per band, per 128-frame tile):
  - scores[t,k] = (2x).e  via PE matmuls. fp32 accuracy at tf32 speed using a
    3-term hi/lo split (a_hi*b_hi + a_hi*b_lo + a_lo*b_hi) in float32r mode
    (1 cyc/row vs 4 for fp32; measured ~1e-6 rel err). The hi/lo split is done
    ON DEVICE (f32->f32r cast rounds to the 11-bit tf32 mantissa; lo = x - hi)
    so x and the codebook ship over DMA once in plain fp32. The 2x scale is
    folded into x host-side.
  - scores -= |e|^2 broadcast (DVE), argmax via DVE max/max_index
    (argmin of distance = argmax of score)
  - gather codeword rows with indirect DMA, PE-transpose to channels-first.
    Transposes/copies for tile i are emitted after the matmuls of tile i+DEFER
    so the PE never stalls on the argmax->gather chain.
  - commit loss via identity sum((q-x)^2) = sum(x^2) - sum(max_score):
    sum(x^2) in f64 on host during input prep; per-frame max scores shipped
    back.
"""
import numpy as np
from contextlib import ExitStack

import concourse.bass as bass
import concourse.tile as tile
from concourse import bacc, mybir
from concourse import bass_utils
from concourse.masks import make_identity

B, NB, D, T, K = 8, 3, 512, 2048, 1024
P = 128
NDC = D // P        # 4 d-chunks of 128
TCH = 1024          # q assembly/output chunk (4KB contiguous output runs)
XCH = 512           # x load/split chunk
KC = 512            # k half (one PSUM bank)
DEFER = 8           # tiles of slack between matmuls and PE transposes

f32 = mybir.dt.float32
f32r = mybir.dt.float32r
u32 = mybir.dt.uint32
i32 = mybir.dt.int32

TRACE = False        # test.py can flip this to capture an NTFF profile
LAST = {}            # test.py introspection (exec_time_ns etc.)


def build_nc(t_total: int = T):
    """Build the per-core Bass program. t_total lets tests build a smaller
    variant for CoreSim."""
    tch = min(TCH, t_total)
    ntt = tch // P            # 128-frame tiles per q chunk
    xch = min(XCH, t_total)   # x load chunk
    nxc = t_total // xch

    nc = bacc.Bacc("TRN2", target_bir_lowering=False, debug=False)

    x_d = nc.dram_tensor("x2", [NB, D, t_total], f32, kind="ExternalInput").ap()
    b_d = nc.dram_tensor("bt", [NB, D, K], f32, kind="ExternalInput").ap()
    e2_d = nc.dram_tensor("e2", [NB, K], f32, kind="ExternalInput").ap()
    cb_d = [
        nc.dram_tensor(f"cb{n}", [K, D], f32, kind="ExternalInput").ap()
        for n in range(NB)
    ]

    q_d = nc.dram_tensor("q", [NB, D, t_total], f32, kind="ExternalOutput").ap()
    codes_d = nc.dram_tensor("codes", [NB, t_total], i32, kind="ExternalOutput").ap()
    smax_d = nc.dram_tensor("smax", [NB, t_total], f32, kind="ExternalOutput").ap()

    x_r = x_d.rearrange("n (c p) t -> p n c t", p=P)
    q_r = q_d.rearrange("n (c p) t -> p n c t", p=P)

    with tile.TileContext(nc) as tc, ExitStack() as ctx:
        const = ctx.enter_context(tc.tile_pool(name="const", bufs=1))
        bspool = ctx.enter_context(tc.tile_pool(name="bspool", bufs=1))
        bpool = ctx.enter_context(tc.tile_pool(name="bpool", bufs=2))
        xspool = ctx.enter_context(tc.tile_pool(name="xspool", bufs=2))
        xpool = ctx.enter_context(tc.tile_pool(name="xpool", bufs=2))
        qpool = ctx.enter_context(tc.tile_pool(name="qpool", bufs=2))
        scpool = ctx.enter_context(tc.tile_pool(name="scpool", bufs=3))
        qrpool = ctx.enter_context(tc.tile_pool(name="qrpool", bufs=10))
        mpool = ctx.enter_context(tc.tile_pool(name="mpool", bufs=8))
        pspool = ctx.enter_context(tc.tile_pool(name="pspool", bufs=3, space="PSUM"))
        pqpool = ctx.enter_context(tc.tile_pool(name="pqpool", bufs=2, space="PSUM"))

        ident = const.tile([P, P], f32)
        make_identity(nc, ident[:])

        e2bc = const.tile([P, NB, K], f32)
        nc.sync.dma_start(e2bc[:], e2_d[None].to_broadcast([P, NB, K]))

        q_tiles = {}      # (band, tci) -> [tile, writes_done]
        pending = []      # deferred (band, tci, tt, qr_tile)

        def flush_one():
            band, tci, tt, qr = pending.pop(0)
            key = (band, tci)
            if key not in q_tiles:
                q_tiles[key] = [
                    qpool.tile([P, NDC, tch], f32, tag="qtc",
                               name=f"qtc_{band}_{tci}"), 0]
            q_tc, done = q_tiles[key]
            pq = pqpool.tile([P, D], f32, tag="pq", name=f"pq_{band}_{tci}_{tt}")
            for c in range(NDC):
                nc.tensor.transpose(
                    pq[:, c * P : (c + 1) * P],
                    qr[:, c * P : (c + 1) * P],
                    ident[:],
                )
            tt0 = tt * P
            nc.scalar.copy(
                out=q_tc[:, :, tt0 : tt0 + P],
                in_=pq[:].rearrange("p (c t) -> p c t", c=NDC),
            )
            q_tiles[key][1] = done + 1
            if q_tiles[key][1] == ntt:
                t0 = tci * tch
                nc.sync.dma_start(q_r[:, band, :, t0 : t0 + tch], q_tc[:])
                del q_tiles[key]

        for band in range(NB):
            # load codebook fp32, split hi/lo on device
            bfp = bspool.tile([P, NDC, K], f32, tag="bfp")
            nc.sync.dma_start(
                bfp[:], b_d[band].rearrange("(c p) k -> p c k", p=P)
            )
            bh_t = bpool.tile([P, NDC, K], f32r, tag="bh")
            nc.vector.tensor_copy(bh_t[:], bfp[:])
            bl_t = bpool.tile([P, NDC, K], f32r, tag="bl")
            nc.vector.tensor_tensor(
                out=bl_t[:], in0=bfp[:], in1=bh_t[:].bitcast(f32),
                op=mybir.AluOpType.subtract,
            )

            for xci in range(nxc):
                x0 = xci * xch
                xfp = xspool.tile([P, NDC, xch], f32, tag="xfp")
                nc.sync.dma_start(xfp[:], x_r[:, band, :, x0 : x0 + xch])
                xh_t = xpool.tile([P, NDC, xch], f32r, tag="xh")
                nc.scalar.copy(xh_t[:], xfp[:])
                xl_t = xpool.tile([P, NDC, xch], f32r, tag="xl")
                nc.vector.tensor_tensor(
                    out=xl_t[:], in0=xfp[:], in1=xh_t[:].bitcast(f32),
                    op=mybir.AluOpType.subtract,
                )

                for tti in range(xch // P):          # 128-frame tiles in x chunk
                    tg = x0 + tti * P                # global t offset
                    tci = tg // tch
                    tt = (tg % tch) // P
                    tt0 = tti * P

                    ps = pspool.tile([P, K], f32, tag="ps")
                    for dc in range(NDC):
                        lh = xh_t[:, dc, tt0 : tt0 + P]
                        ll = xl_t[:, dc, tt0 : tt0 + P]
                        st = dc == 0
                        sp = dc == NDC - 1
                        nc.tensor.matmul(ps[:, 0:KC], lh, bh_t[:, dc, 0:KC],
                                         start=st, stop=False)
                        nc.tensor.matmul(ps[:, KC:K], lh, bh_t[:, dc, KC:K],
                                         start=st, stop=False)
                        nc.tensor.matmul(ps[:, 0:KC], lh, bl_t[:, dc, 0:KC],
                                         start=False, stop=False)
                        nc.tensor.matmul(ps[:, KC:K], lh, bl_t[:, dc, KC:K],
                                         start=False, stop=False)
                        nc.tensor.matmul(ps[:, 0:KC], ll, bh_t[:, dc, 0:KC],
                                         start=False, stop=sp)
                        nc.tensor.matmul(ps[:, KC:K], ll, bh_t[:, dc, KC:K],
                                         start=False, stop=sp)

                    sc = scpool.tile([P, K], f32, tag="sc")
                    nc.vector.tensor_tensor(
                        out=sc[:], in0=ps[:], in1=e2bc[:, band, :],
                        op=mybir.AluOpType.subtract,
                    )
                    mx = mpool.tile([P, 8], f32, tag="mx")
                    mi = mpool.tile([P, 8], u32, tag="mi")
                    nc.vector.max(mx[:], sc[:])
                    nc.vector.max_index(mi[:], mx[:], sc[:])

                    nc.sync.dma_start(
                        codes_d[band, tg : tg + P, None], mi[:, 0:1].bitcast(i32)
                    )
                    nc.sync.dma_start(smax_d[band, tg : tg + P, None], mx[:, 0:1])

                    qr = qrpool.tile([P, D], f32, tag="qr")
                    nc.gpsimd.indirect_dma_start(
                        out=qr[:],
                        out_offset=None,
                        in_=cb_d[band],
                        in_offset=bass.IndirectOffsetOnAxis(ap=mi[:, 0:1], axis=0),
                    )

                    pending.append((band, tci, tt, qr))
                    if len(pending) > DEFER:
                        flush_one()

        while pending:
            flush_one()

    nc.compile()
    return nc


_NC_CACHE = {}


def _get_nc(t_total: int = T):
    if t_total not in _NC_CACHE:
        _NC_CACHE[t_total] = build_nc(t_total)
    return _NC_CACHE[t_total]


def prep_inputs(x: np.ndarray, codebook: np.ndarray):
    """Host-side prep: 2x scale, codebook transpose, |e|^2, per-core maps,
    sum(x^2)."""
    x = np.ascontiguousarray(x, dtype=np.float32)
    codebook = np.ascontiguousarray(codebook, dtype=np.float32)

    cbt = np.ascontiguousarray(codebook.transpose(0, 2, 1))  # [NB, D, K]
    e2 = (codebook.astype(np.float64) ** 2).sum(-1).astype(np.float32)  # [NB, K]

    shared = {"bt": cbt, "e2": e2}
    for n in range(NB):
        shared[f"cb{n}"] = np.ascontiguousarray(codebook[n])

    in_maps = []
    for b in range(B):
        in_maps.append({"x2": 2.0 * x[b], **shared})

    sum_x2 = float((x.astype(np.float64) ** 2).sum())
    return in_maps, sum_x2


def kernel(x: np.ndarray, codebook: np.ndarray):
    x = np.asarray(x)
    codebook = np.asarray(codebook)
    in_maps, sum_x2 = prep_inputs(x, codebook)
    nc = _get_nc()

    res = bass_utils.run_bass_kernel_spmd(
        nc, in_maps, core_ids=list(range(B)), trace=TRACE
    )
    LAST["exec_time_ns"] = res.exec_time_ns
    LAST["profile_json"] = res.profile_json

    outs = res.results
    quantized = np.stack([outs[b]["q"] for b in range(B)])        # [B,NB,D,T]
    codes = np.stack([outs[b]["codes"] for b in range(B)])        # [B,NB,T]
    smax = np.stack([outs[b]["smax"] for b in range(B)])          # [B,NB,T]

    sum_smax = float(smax.astype(np.float64).sum())
    commit_loss = np.float32((sum_x2 - sum_smax) / (B * NB * T * D))
    return quantized, codes, commit_loss
